# revision 25
# baseline (speedup 1.0000x reference)
"""Grouped SwiGLU experts (MoE post-dispatch compute) on 8 Trainium2 cores.

Expert-parallel with optional hidden-dim tensor parallelism: host gathers
tokens per expert (the "all-to-all dispatch") and packs them into a uniform
per-core slot schedule (specialized to the actual counts at compile time).
A slot is a weight stream for one expert covering either the full hidden dim
(8 h-chunks) or half of it (4 h-chunks); half-width slots halve the weight
traffic per core and their partial outputs are summed on the host. Each slot
runs  hT = silu(w1.T x.T) * (w3.T x.T);  out = (hT.T) @ w2  in bf16 with
fp32 PSUM accumulation, and the host scatters rows back to token positions.
"""

import itertools
import numpy as np
import ml_dtypes

# ---- problem constants (from the reference module) ----
T, D, H, E, R, ALIGN = 8192, 4096, 1024, 8, 2, 16
P = 128          # partition width
DT = D // P      # 32 d-tiles
HT = H // P      # 8 h-chunks (full width)
NCORES = 8
UNIT = 8         # scheduling granularity in rows
MAX_UNITS = 64   # max rows per slot = 512 (psum bank / f32 free-dim cap)

BF16 = ml_dtypes.bfloat16

# cost model for the planner
HU_NS = 1280          # compute ns per half-unit (8 rows x 384 cyc @2.4GHz)
PLAN_BW = 330.0       # planning DMA bandwidth, bytes/ns
W_FULL = 3 * D * H * 2            # full weight stream bytes (25.2MB)
W_HALF = W_FULL // 2
ROW_IO = 2 * D * 2                # x + out bytes per scheduled row (bf16)


def _permute_indices(counts):
    """numpy port of reference._permute_indices."""
    counts = counts.astype(np.int64)
    max_len = T + E * ALIGN
    start_index = np.cumsum(counts) - counts
    total = counts.reshape(R, E).sum(0)
    m_sizes = ((np.maximum(total, ALIGN) + ALIGN - 1) // ALIGN * ALIGN).astype(np.int64)
    m_offsets = np.cumsum(m_sizes)
    write_offsets = m_offsets - m_sizes
    c_er = counts.reshape(R, E).T
    seg_ws = (write_offsets[:, None] + np.cumsum(c_er, 1) - c_er).reshape(-1)
    seg_len = c_er.reshape(-1)
    seg_src = start_index.reshape(R, E).T.reshape(-1)
    pos = np.arange(max_len, dtype=np.int64)
    idx = np.clip(np.searchsorted(seg_ws, pos, side="right") - 1, 0, E * R - 1)
    within = pos - seg_ws[idx]
    valid = (within >= 0) & (within < seg_len[idx])
    perm = np.where(valid, seg_src[idx] + within, T)
    return perm.astype(np.int64), m_sizes, (m_offsets - m_sizes)


def _ffd(tasks, bins, strict, order=0, pick=0):
    """Pack tasks (key, size) into bins [cap, core, j], splitting freely.
    Single task piece per bin. Returns (asg {(core,j): (key, u0, nu)},
    leftovers {key: units}); if strict, returns None on leftover."""
    bins = sorted(bins, key=lambda b: -b[0])
    used = [False] * len(bins)
    asg = {}
    left = {}
    tasks = [t for t in tasks if t[1] > 0]
    for key, size in sorted(tasks, key=lambda t: t[1] if order else -t[1]):
        rem = int(size)
        u0 = 0
        while rem > 0:
            best_le, best_gt = None, None
            for i, (cap, c, j) in enumerate(bins):
                if used[i]:
                    continue
                if cap <= rem:
                    best_le = i  # bins desc: first such is largest
                    break
                best_gt = i  # keeps updating: last seen = smallest so far
            if pick:  # prefer tight fit over fill-fully
                i = best_gt if best_gt is not None else best_le
            else:
                i = best_le if best_le is not None else best_gt
            if i is None:
                if strict:
                    return None
                left[key] = rem
                break
            cap, c, j = bins[i]
            used[i] = True
            nb = min(cap, rem)
            asg[(c, j)] = (key, u0, nb)
            u0 += nb
            rem -= nb
    return asg, left


def _try_config(slots, nunits):
    """slots: [(units, nhc)]. Returns asg {(c,j): (e, half, u0, nu)} with
    half=None for full-width slots, or None if infeasible."""
    full_bins = [[u, c, j] for c in range(NCORES)
                 for j, (u, nhc) in enumerate(slots) if nhc == HT]
    half_bins = [[u, c, j] for c in range(NCORES)
                 for j, (u, nhc) in enumerate(slots) if nhc == HT // 2]

    def finish(asg_f, left):
        left = {e: r for e, r in left.items() if r > 0}
        if left and not half_bins:
            return None
        # leftover of expert e needs BOTH halves scheduled (split freely)
        half_tasks = [((e, h), r) for e, r in left.items() for h in (0, 1)]
        r = ({}, {})
        if half_tasks:
            r = None
            for order, pick in ((0, 0), (0, 1), (1, 0)):
                r = _ffd(half_tasks, half_bins, True, order, pick)
                if r is not None:
                    break
        if r is None:
            return None
        asg_h, _ = r
        # full and half pieces of expert e cover disjoint unit ranges:
        # full pieces cover [0, nf_e), halves [nf_e, n_e)
        nf = {e: 0 for e in range(E)}
        for (c, j), (e, u0, nu) in asg_f.items():
            nf[e] = max(nf[e], u0 + nu)
        asg = {}
        for (c, j), (e, u0, nu) in asg_f.items():
            asg[(c, j)] = (e, None, u0, nu)
        for (c, j), ((e, h), u0, nu) in asg_h.items():
            asg[(c, j)] = (e, h, nf[e] + u0, nu)
        return asg

    if not full_bins:
        return finish({}, {e: int(nunits[e]) for e in range(E)})
    # which experts to route (primarily) through full-width bins is a small
    # subset-selection problem; greedy FFD alone misses exact packings
    fullcap = sum(b[0] for b in full_bins)
    subsets = []
    for mask in range(1 << E):
        S = [e for e in range(E) if mask >> e & 1]
        tot = sum(int(nunits[e]) for e in S)
        subsets.append((abs(tot - fullcap), mask, S))
    subsets.sort()
    for _, mask, S in subsets[:24]:
        tasks = [(e, int(nunits[e])) for e in S]
        for order, pick in ((0, 0), (0, 1)):
            asg_f, left = _ffd(tasks, full_bins, False, order, pick)
            for e in range(E):
                if not (mask >> e & 1):
                    left[e] = int(nunits[e])
            asg = finish(asg_f, left)
            if asg is not None:
                return asg
    return None


def _cost(slots):
    chu = sum(u * (2 if nhc == HT else 1) for u, nhc in slots)
    rows = sum(u for u, _ in slots) * UNIT
    wbytes = sum(W_FULL if nhc == HT else W_HALF for _, nhc in slots)
    dma_ns = (wbytes + rows * ROW_IO) / PLAN_BW + 8000
    comp_ns = chu * HU_NS + 12000
    return max(comp_ns, dma_ns), dma_ns, len(slots)


def _plan(nunits):
    """Returns (slots [(units, nhc)], asg {(c,j): (e, half, u0, nu)})."""
    total_hu = 2 * int(sum(nunits))
    need = -(-total_hu // NCORES)
    best = None

    def consider(cfg):
        nonlocal best
        c = _cost(cfg)
        if best is not None and c >= best[0]:
            return
        asg = _try_config(cfg, nunits)
        if asg is not None:
            best = (c, list(cfg), asg)

    import time
    deadline = time.monotonic() + 8.0
    for extra_cap in (8, 24, 2 * need + 8):
        hi = need + extra_cap
        cands = []
        for ns in (1, 2, 3):
            for widths in itertools.product((HT, HT // 2), repeat=ns):
                mults = [2 if w == HT else 1 for w in widths]
                if ns == 1:
                    for u1 in range(1, MAX_UNITS + 1):
                        if need <= mults[0] * u1 <= hi:
                            cands.append(((u1, widths[0]),))
                    continue
                for u1 in range(MAX_UNITS, 0, -1):
                    if ns == 2:
                        lo2 = -(-(need - mults[0] * u1) // mults[1])
                        hi2 = (hi - mults[0] * u1) // mults[1]
                        hi2 = min(hi2, u1 if widths[1] == widths[0] else MAX_UNITS)
                        for u2 in range(max(1, lo2), hi2 + 1):
                            cands.append(((u1, widths[0]), (u2, widths[1])))
                        continue
                    hi2 = u1 if widths[1] == widths[0] else MAX_UNITS
                    for u2 in range(hi2, 0, -1):
                        base = mults[0] * u1 + mults[1] * u2
                        lo3 = -(-(need - base) // mults[2])
                        hi3 = (hi - base) // mults[2]
                        hi3 = min(hi3, u2 if widths[2] == widths[1] else MAX_UNITS)
                        for u3 in range(max(1, lo3), hi3 + 1):
                            cands.append(((u1, widths[0]), (u2, widths[1]),
                                          (u3, widths[2])))
        # cost is config-level (independent of the assignment), so the
        # first feasible config in cost order is optimal
        cands.sort(key=_cost)
        for cfg in cands[:4000]:
            if time.monotonic() > deadline:
                break
            asg = _try_config(cfg, nunits)
            if asg is not None:
                best = (_cost(cfg), list(cfg), asg)
                break
        if best is not None:
            break
    if best is None:
        return _fallback_plan(nunits)
    return best[1], best[2]


def _fallback_plan(nunits):
    """Constructive guaranteed plan: half-width tasks occupy whole 64-unit
    bins (last bin per task partial). Works for any counts."""
    tasks = [(e, h, int(nunits[e])) for e in range(E) for h in (0, 1)
             if nunits[e] > 0]
    nbins = sum(-(-sz // MAX_UNITS) for _, _, sz in tasks)
    ns = -(-nbins // NCORES)
    slots = [(MAX_UNITS, HT // 2)] * ns
    asg = {}
    bins = [(c, j) for j in range(ns) for c in range(NCORES)]
    bi = 0
    for e, h, sz in tasks:
        u0 = 0
        while u0 < sz:
            nu = min(MAX_UNITS, sz - u0)
            asg[bins[bi]] = (e, h, u0, nu)
            bi += 1
            u0 += nu
    return slots, asg


def _build_program(slots):
    import concourse.mybir as mybir
    import concourse.tile as tile
    from concourse import bacc

    bf = mybir.dt.bfloat16
    f32 = mybir.dt.float32
    SILU = mybir.ActivationFunctionType.Silu
    COPY = mybir.ActivationFunctionType.Copy

    nc = bacc.Bacc("TRN2", target_bir_lowering=False, debug=False,
                   num_devices=NCORES)

    XC = DT // 4  # d-tiles per xt chunk

    xt_d, w13_d, w2_d, out_d = [], [], [], []
    for j, (U, NHC) in enumerate(slots):
        M = U * UNIT
        xt_d.append(nc.dram_tensor(f"xt{j}", [4, P, XC * M], bf, kind="ExternalInput"))
        w13_d.append(nc.dram_tensor(f"w13p{j}", [2, NHC, P, D], bf, kind="ExternalInput"))
        w2_d.append(nc.dram_tensor(f"w2p{j}", [NHC, P, D], bf, kind="ExternalInput"))
        # output is stored transposed: out[k, p, t*M+m] = y[m, (k*XC+t)*P+p]
        out_d.append(nc.dram_tensor(f"out{j}", [4, P, XC * M], bf, kind="ExternalOutput"))

    with tile.TileContext(nc) as tc:
        with (
            tc.tile_pool(name="xt", bufs=8) as xt_pool,
            tc.tile_pool(name="wp", bufs=8) as wp_pool,
            tc.tile_pool(name="w2", bufs=6) as w2_pool,
            tc.tile_pool(name="ht", bufs=12) as ht_pool,
            tc.tile_pool(name="stmp", bufs=2) as stmp_pool,
            tc.tile_pool(name="ost", bufs=2) as ost_pool,
            tc.tile_pool(name="ps1", bufs=2, space="PSUM") as ps1_pool,
            tc.tile_pool(name="ps3", bufs=2, space="PSUM") as ps3_pool,
            tc.tile_pool(name="pso", bufs=3, space="PSUM") as pso_pool,
            tc.tile_pool(name="warm", bufs=1) as warm_pool,
        ):
            # keep the PE busy (HAM at K=8/8) while the first real DMAs land
            wz = warm_pool.tile([P, P], bf, tag="warm", name="warmz")
            nc.gpsimd.memset(wz[:], 0.0)
            pw = pso_pool.tile([P, P], f32, tag="pso", name="warmp")
            for _ in range(96):
                nc.tensor.matmul(pw[:], wz[:], wz[:], start=True, stop=True)

            for j, (U, NHC) in enumerate(slots):
                M = U * UNIT

                def load_wpair(hc, j=j, split=False):
                    # split=True halves the first panels so the very first
                    # matmuls wait on 0.5MB, not 1MB
                    DH = D // 2 if split else D
                    segs = ([], [])
                    for h0 in range(0, D, DH):
                        for i in (0, 1):
                            t = wp_pool.tile([P, DH], bf, tag="wp",
                                             name=f"w{3 if i else 1}_{j}_{hc}_{h0}")
                            nc.sync.dma_start(out=t[:],
                                              in_=w13_d[j][i, hc, :, h0:h0 + DH])
                            segs[i].append(t)
                    return (segs[0], segs[1], DH)

                # critical path first: hc=0 weight panels, then token chunks,
                # then 2 more prefetched panel pairs (deep prefetch so the
                # in-order sync dispatcher never starves the PE);
                # w2 (phase 2 only) is deferred until after hc=1 emission
                wq = [load_wpair(0, split=(j == 0))]
                xts = []
                for k in range(4):
                    t = xt_pool.tile([P, XC * M], bf, tag="xt", name=f"xt{j}_{k}")
                    nc.sync.dma_start(out=t[:], in_=xt_d[j][k])
                    xts.append(t)
                for hc in (1, 2):
                    if hc < NHC:
                        wq.append(load_wpair(hc))
                hts = [ht_pool.tile([P, M], bf, tag="ht", name=f"ht{j}_{h}")
                       for h in range(NHC)]
                w2s = None
                for hc in range(NHC):
                    w1segs, w3segs, DH = wq[hc]
                    if hc + 3 < NHC:
                        wq.append(load_wpair(hc + 3))
                    ps1 = ps1_pool.tile([P, M], f32, tag="ps1")
                    ps3 = ps3_pool.tile([P, M], f32, tag="ps3")
                    for d in range(DT):
                        xa = xts[d // XC][:, (d % XC) * M:(d % XC + 1) * M]
                        si, sc = (d * P) // DH, (d * P) % DH
                        nc.tensor.matmul(ps1[:], w1segs[si][:, sc:sc + P],
                                         xa, start=(d == 0), stop=(d == DT - 1))
                        nc.tensor.matmul(ps3[:], w3segs[si][:, sc:sc + P],
                                         xa, start=(d == 0), stop=(d == DT - 1))
                    tmp = stmp_pool.tile([P, M], f32, tag="stmp")
                    nc.scalar.activation(tmp[:], ps1[:], SILU)
                    nc.vector.tensor_mul(hts[hc][:], tmp[:], ps3[:])
                    if hc == min(1, NHC - 1):
                        w2s = []
                        for h in range(NHC):
                            t = w2_pool.tile([P, D], bf, tag="w2",
                                             name=f"w2_{j}_{h}")
                            nc.sync.dma_start(out=t[:], in_=w2_d[j][h])
                            w2s.append(t)
                # phase 2, transposed: stationary w2 d-tile, stream tokens.
                # out psum is [d-cols, tokens]; no 128-row block padding.
                ob = None
                for d in range(DT):
                    pot = pso_pool.tile([P, M], f32, tag="pso")
                    for h in range(NHC):
                        nc.tensor.matmul(pot[:], w2s[h][:, d * P:(d + 1) * P],
                                         hts[h][:], start=(h == 0),
                                         stop=(h == NHC - 1))
                    if d % XC == 0:
                        ob = ost_pool.tile([P, XC * M], bf, tag="ost")
                    t = d % XC
                    # split psum->sbuf casts across scalar and vector
                    if d % 2 == 0:
                        nc.scalar.activation(ob[:, t * M:(t + 1) * M], pot[:], COPY)
                    else:
                        nc.vector.tensor_copy(ob[:, t * M:(t + 1) * M], pot[:])
                    if t == XC - 1:
                        nc.sync.dma_start(out=out_d[j][d // XC], in_=ob[:])

    nc.compile()
    return nc


_CACHE = {}


def _get_program(slots):
    key = tuple(slots)
    if key not in _CACHE:
        _CACHE[key] = _build_program(slots)
    return _CACHE[key]


_LAST_RESULT = None


def kernel(x, w1, w2, w3, num_tokens_per_expert):
    import os
    from concourse.bass_utils import run_bass_kernel_spmd

    x = np.asarray(x, dtype=np.float32)
    counts = np.asarray(num_tokens_per_expert).astype(np.int64)
    perm, m_sizes, m_off = _permute_indices(counts)
    # schedule only the real rows (m_sizes tail padding computes to zero
    # and is dropped anyway), rounded up to UNIT
    totals = counts.reshape(R, E).sum(0)
    nunits = (totals + UNIT - 1) // UNIT

    slots, asg = _plan(nunits)
    nc = _get_program(slots)

    # expert-grouped token stream (the dispatch): rows of x per expert
    x_pad = np.concatenate([x, np.zeros((1, D), np.float32)], axis=0)
    ltot = int(m_sizes.sum())
    xp = x_pad[perm[:ltot]].astype(BF16)  # [ltot, D] expert-grouped
    xe = [xp[m_off[e]:m_off[e] + m_sizes[e]] for e in range(E)]

    w1b = [np.ascontiguousarray(
        np.asarray(w1[e], np.float32).reshape(DT, P, HT, P)
        .transpose(2, 1, 0, 3).reshape(HT, P, D)).astype(BF16) for e in range(E)]
    w3b = [np.ascontiguousarray(
        np.asarray(w3[e], np.float32).reshape(DT, P, HT, P)
        .transpose(2, 1, 0, 3).reshape(HT, P, D)).astype(BF16) for e in range(E)]
    w2b = [np.asarray(w2[e], np.float32).astype(BF16).reshape(HT, P, D)
           for e in range(E)]

    XC = DT // 4
    w13_cache = {}

    def w13_for(e, half, nhc):
        key = (e, half)
        if key not in w13_cache:
            off = 0 if half is None else half * nhc
            w13_cache[key] = np.stack([w1b[e][off:off + nhc],
                                       w3b[e][off:off + nhc]])
        return w13_cache[key]

    in_maps = []
    for c in range(NCORES):
        mm = {}
        for j, (U, NHC) in enumerate(slots):
            M = U * UNIT
            ent = asg.get((c, j))
            e, half = (ent[0], ent[1]) if ent is not None else (0, None if NHC == HT else 0)
            blk = np.zeros((M, D), BF16)
            if ent is not None:
                _, _, u0, nu = ent
                blk[:nu * UNIT] = xe[e][u0 * UNIT:(u0 + nu) * UNIT]
            # xt[k][p, t*M+m] = blk[m, (k*XC+t)*128+p]
            mm[f"xt{j}"] = np.ascontiguousarray(
                blk.reshape(M, 4, XC, P).transpose(1, 3, 2, 0).reshape(4, P, XC * M))
            mm[f"w13p{j}"] = w13_for(e, half, NHC)
            off = 0 if half is None else half * NHC
            mm[f"w2p{j}"] = w2b[e][off:off + NHC]
        in_maps.append(mm)

    kw = {}
    if os.environ.get("KERNEL_TRACE"):
        kw = dict(trace=True, tmpdir=os.environ.get("KERNEL_TRACE_DIR") or None)
    res = run_bass_kernel_spmd(nc, in_maps, core_ids=list(range(NCORES)), **kw)
    global _LAST_RESULT
    _LAST_RESULT = res

    # reassemble expert-grouped output stream (summing half partials),
    # then scatter to token order
    outp = np.zeros((ltot, D), np.float32)
    for (c, j), (e, half, u0, nu) in asg.items():
        nr = nu * UNIT
        M = slots[j][0] * UNIT
        # out[k, p, t*M+m] = y[m, (k*XC+t)*P+p] -> [M, D]
        seg = np.asarray(res.results[c][f"out{j}"], np.float32) \
            .reshape(4, P, XC, M).transpose(3, 0, 2, 1).reshape(M, D)
        outp[m_off[e] + u0 * UNIT:m_off[e] + u0 * UNIT + nr] += seg[:nr]

    out = np.zeros((T + 1, D), np.float32)
    out[perm[:ltot]] = outp
    return out[:T]
